# revision 1
# baseline (speedup 1.0000x reference)
"""Trainium2 Bass kernel for nn_Attention_75093208203309 (sparse attention).

Contract: kernel(**inputs) takes FULL unsharded inputs (numpy), returns the
FULL [4096, 1024] float32 output. Internally shards query rows across 8
NeuronCores; k/v are computed locally per-core and all-gathered on-device.

Layout strategy (all transposes done host-side in numpy):
  - Per core i (rows = 512*i .. 512*(i+1)):
      qT, qcT   [D, 512]   computed on device from xT shard (f32r matmuls)
      kT_local  [D, 512] -> AllGather -> zk  (kT of all rows)
      v_local   [512, D] -> AllGather -> zv  (v natural layout)
      S.T tiles [mk=128, m=512] = k @ qT : lhsT = kT slice (shared stationary
        operand with conn.T = k @ qcT), rhs = qT / qcT.
      uint8 masks decoded on DVE; the straight-through hard-sigmoid is a
        single is_gt against -bias on the raw conn logits; exp uses a fixed
        -10000 shift (class-2 mask entries dominate every row, so no row max
        is needed); softmax normalization applied after O = E @ v using
        per-partition reciprocal sums. All matmuls run as float32r (PE reads
        truncate operands to 11 mantissa bits); E and v are bf16.
"""

import contextlib

import numpy as np
import ml_dtypes  # noqa: F401  (np bfloat16 views)

import concourse.bass as bass
import concourse.bacc as bacc
import concourse.mybir as mybir
import concourse.tile as tile
from concourse import bass_utils

f32 = mybir.dt.float32
f32r = mybir.dt.float32r
bf16 = mybir.dt.bfloat16
AF = mybir.ActivationFunctionType
ALU = mybir.AluOpType

NCORES = 8
N, D = 4096, 1024
M = N // NCORES          # 512 rows per core
MT = M // 128            # 4 m-tiles
G = N // 128             # 32 mk-tiles
DC = D // 128            # 8 contraction tiles
MSCALE = 320000.0        # 10000 * 32 (folds softmax scale 1/sqrt(D)=1/32)
RG = [list(range(NCORES))]


def build(bias_val: float, timing_mode: bool = False, repeats: int = 1):
    """timing_mode: single-core variant with zk/zv as ExternalInputs and no
    collectives, for TimelineSim cost-model profiling."""
    nc = bacc.Bacc(None, num_devices=NCORES, debug=False)

    xt = nc.dram_tensor("xt", [DC, 128, M], f32, kind="ExternalInput")
    xn = nc.dram_tensor("xn", [MT, 128, D], f32, kind="ExternalInput")
    wqt = nc.dram_tensor("wqt", [DC, 128, D], f32, kind="ExternalInput")
    wkt = nc.dram_tensor("wkt", [DC, 128, D], f32, kind="ExternalInput")
    wvt = nc.dram_tensor("wvt", [DC, 128, D], f32, kind="ExternalInput")
    cn = nc.dram_tensor("cn", [DC, 128, D], f32, kind="ExternalInput")
    bq = nc.dram_tensor("bq", [128, DC], f32, kind="ExternalInput")
    bk = nc.dram_tensor("bk", [128, DC], f32, kind="ExternalInput")
    bv = nc.dram_tensor("bv", [1, D], f32, kind="ExternalInput")
    bnd = nc.dram_tensor("bnd", [MT, 128, 1], f32, kind="ExternalInput")
    amh = nc.dram_tensor("amh", [G, 128, M], mybir.dt.uint8, kind="ExternalInput")
    lmh = nc.dram_tensor("lmh", [G, 128, M], mybir.dt.uint8, kind="ExternalInput")
    ones8 = nc.dram_tensor("ones8", [128, 8], mybir.dt.bfloat16, kind="ExternalInput")
    ones1 = nc.dram_tensor("ones1", [1, 128], f32, kind="ExternalInput")
    out = nc.dram_tensor("out", [MT, 128, D], f32, kind="ExternalOutput")

    with tile.TileContext(nc) as tc, contextlib.ExitStack() as ST:
        pp = ST.enter_context(tc.tile_pool(name="persist", bufs=1))
        dp = ST.enter_context(tc.tile_pool(name="dram", bufs=1, space="DRAM"))

        ones_s = pp.tile([128, 8], bf16, name="ones_s")
        onesk1 = pp.tile([1, 128], f32r, name="onesk1")
        bq_s = pp.tile([128, DC], f32, name="bq_s")
        bk_s = pp.tile([128, DC], f32, name="bk_s")
        bv_s = pp.tile([1, D], f32r, name="bv_s")
        bnd_s = pp.tile([128, MT], f32, name="bnd_s")
        recip_s = pp.tile([128, MT], f32, name="recip_s")
        s1_s = pp.tile([128, MT], f32, name="s1_s")
        omb_s = pp.tile([128, MT], f32, name="omb_s")
        shift_s = pp.tile([128, 1], f32, name="shift_s")
        nc.vector.memset(shift_s[:], -20000.0)

        nc.sync.dma_start(ones_s[:], ones8.ap())
        nc.sync.dma_start(onesk1[:], ones1.ap().bitcast(f32r))
        nc.sync.dma_start(bq_s[:], bq.ap())
        nc.sync.dma_start(bk_s[:], bk.ap())
        nc.sync.dma_start(bv_s[:], bv.ap().bitcast(f32r))
        for mt in range(MT):
            nc.sync.dma_start(bnd_s[:, mt : mt + 1], bnd.ap()[mt])
        nc.vector.tensor_scalar(omb_s[:], bnd_s[:], -1.0, 1.0, ALU.mult, ALU.add)

        if timing_mode:
            zk = nc.dram_tensor("zk", [NCORES, DC, 128, M], f32,
                                kind="ExternalInput").ap()
            zv = nc.dram_tensor("zv", [NCORES, MT, 128, D], bf16,
                                kind="ExternalInput").ap()

        for _rep in range(repeats):
            kt_loc = dp.tile([DC, 128, M], f32, name=f"kt_loc{_rep}")
            v_loc = dp.tile([MT, 128, D], bf16, name=f"v_loc{_rep}")
            if not timing_mode:
                zk = dp.tile([NCORES, DC, 128, M], f32, name=f"zk{_rep}",
                             addr_space="Shared")
                zv = dp.tile([NCORES, MT, 128, D], bf16, name=f"zv{_rep}",
                             addr_space="Shared")
            E3 = [
                pp.tile([128, M], bf16, tag="E3", name=f"E3_{g}_{_rep}", bufs=G)
                for g in range(G)
            ]
            # pools whose lifetimes cross phase boundaries, closed manually
            q_stack = contextlib.ExitStack()
            qp = q_stack.enter_context(tc.tile_pool(name="qpool", bufs=1))
            kp = q_stack.enter_context(tc.tile_pool(name="s_kt", bufs=3))
            qt_s = qp.tile([128, DC, M], f32r, name="qt_s")
            qct_s = qp.tile([128, DC, M], f32r, name="qct_s")

            ktb_pre = {}

            def load_ktb(j):
                ktb = kp.tile([128, DC, M], f32r, tag="kt", name="ktb")
                nc.sync.dma_start(
                    ktb[:], zk[j].rearrange("t p m -> p t m").bitcast(f32r)
                )
                ktb_pre[j] = ktb
                return ktb

            # ---------------- QKV projections (t-outer) ----------------
            with (
                tc.tile_pool(name="qkv_w", bufs=3) as wp,
                tc.tile_pool(name="qkv_x", bufs=1) as xp,
                tc.tile_pool(name="qkv_sb", bufs=3) as sp,
                tc.tile_pool(name="qkv_ps", bufs=8, space="PSUM") as ps1,
            ):
                xt_s = xp.tile([128, DC, M], f32r, name="xt_s")
                for t in range(DC):
                    nc.sync.dma_start(xt_s[:, t, :], xt.ap()[t].bitcast(f32r))

                def load_w_half(wdram, half, name):
                    w_h = wp.tile([128, DC, 512], f32r, tag="w", name=f"w_{name}{half}")
                    for t in range(DC):
                        nc.sync.dma_start(
                            w_h[:, t, :],
                            wdram.ap()[t][:, half * 512 : (half + 1) * 512]
                            .bitcast(f32r),
                        )
                    return w_h

                def mm_half(w_h, rhs_tile, psums):
                    for t in range(DC):
                        for oi in range(4):
                            nc.tensor.matmul(
                                psums[oi][:],
                                w_h[:, t, oi * 128 : (oi + 1) * 128],
                                rhs_tile[:, t, :],
                                start=(t == 0),
                                stop=(t == DC - 1),
                            )

                # kT first: it feeds the first all-gather. Each phase's weight
                # DMAs are emitted before the previous phase's store epilogue
                # so store deps never head-of-line-block the weight stream.
                wk_h = [load_w_half(wkt, h, "k") for h in range(2)]
                kpss = []
                for half in range(2):
                    kps = [
                        ps1.tile([128, M], f32, tag="ps1", name=f"kps{half}{i}")
                        for i in range(4)
                    ]
                    mm_half(wk_h[half], xt_s, kps)
                    kpss.append(kps)
                wv_h = [load_w_half(wvt, h, "v") for h in range(2)]
                for half in range(2):
                    for oi in range(4):
                        ot = half * 4 + oi
                        kt_sb = sp.tile([128, M], f32, tag="kvsb", name="kt_sb")
                        nc.scalar.activation(
                            kt_sb[:], kpss[half][oi][:], AF.Identity,
                            bias=bk_s[:, ot : ot + 1],
                        )
                        nc.sync.dma_start(kt_loc[ot], kt_sb[:])
                if not timing_mode:
                    nc.gpsimd.collective_compute(
                        "AllGather", ALU.bypass, replica_groups=RG,
                        ins=[kt_loc[:].opt()], outs=[zk[:].opt()],
                    )
                load_ktb(0)

                # v: halves are the d-halves directly
                vpss = []
                for dh in range(2):
                    vps = [
                        ps1.tile([128, 512], f32, tag="ps1", name=f"vps{dh}{mt}")
                        for mt in range(MT)
                    ]
                    for t in range(DC):
                        for mt in range(MT):
                            nc.tensor.matmul(
                                vps[mt][:],
                                xt_s[:, t, mt * 128 : (mt + 1) * 128],
                                wv_h[dh][:, t, :],
                                start=(t == 0),
                                stop=False,
                            )
                    vpss.append(vps)
                wq_h = [load_w_half(wqt, h, "q") for h in range(2)]
                for dh in range(2):
                    for mt in range(MT):
                        nc.tensor.matmul(
                            vpss[dh][mt][:],
                            onesk1[:, :],
                            bv_s[:, dh * 512 : (dh + 1) * 512],
                            start=False,
                            stop=True,
                        )
                        v_sb = sp.tile([128, 512], bf16, tag="kvsb", name="v_sb")
                        nc.scalar.copy(v_sb[:], vpss[dh][mt][:])
                        nc.sync.dma_start(
                            v_loc[mt, :, dh * 512 : (dh + 1) * 512], v_sb[:]
                        )
                if not timing_mode:
                    nc.gpsimd.collective_compute(
                        "AllGather", ALU.bypass, replica_groups=RG,
                        ins=[v_loc[:].opt()], outs=[zv[:].opt()],
                    )

                qpss = []
                for half in range(2):
                    qps = [
                        ps1.tile([128, M], f32, tag="ps1", name=f"qps{half}{i}")
                        for i in range(4)
                    ]
                    mm_half(wq_h[half], xt_s, qps)
                    qpss.append(qps)
                cn_h = [load_w_half(cn, h, "c") for h in range(2)]
                for half in range(2):
                    for oi in range(4):
                        ot = half * 4 + oi
                        nc.scalar.activation(
                            qt_s[:, ot, :], qpss[half][oi][:], AF.Identity,
                            bias=bq_s[:, ot : ot + 1],
                        )

                for half in range(2):
                    cps = [
                        ps1.tile([128, M], f32, tag="ps1", name=f"cps{half}{i}")
                        for i in range(4)
                    ]
                    mm_half(cn_h[half], qt_s, cps)
                    for oi in range(4):
                        ot = half * 4 + oi
                        nc.scalar.copy(qct_s[:, ot, :], cps[oi][:])

            # v tiles + xn survive into the O phase
            o_stack = contextlib.ExitStack()
            vpool = o_stack.enter_context(
                tc.tile_pool(name="o_v", bufs=3, side="right")
            )
            xop = o_stack.enter_context(
                tc.tile_pool(name="o_x", bufs=1, side="right")
            )
            xn_s = xop.tile([128, MT, D], f32, name="xn_s")
            nc.sync.dma_start(xn_s[:], xn.ap().rearrange("m p d -> p m d"))
            vt_pre = {}

            def load_vt(dh, j):
                vt = vpool.tile([128, 4, 512], bf16, tag="v", name="vt", bufs=4)
                for vb in range(4):
                    nc.sync.dma_start(
                        vt[:, vb, :], zv[j][vb][:, dh * 512 : (dh + 1) * 512]
                    )
                vt_pre[(dh, j)] = vt
                return vt

            # ---------------- S phase: logits, mask, exp ----------------
            with (
                tc.tile_pool(name="s_m", bufs=12) as mp,
                tc.tile_pool(name="s_t", bufs=4) as tpool,
                tc.tile_pool(name="s_psA", bufs=3, space="PSUM") as psA,
                tc.tile_pool(name="s_psB", bufs=3, space="PSUM") as psB,
            ):
                load_vt(0, 0)
                for j in range(NCORES):
                    ktb = ktb_pre.pop(j, None) or load_ktb(j)
                    ktb_pre.pop(j, None)
                    for gi in range(4):
                        g = j * 4 + gi
                        B = psB.tile([128, M], f32, tag="B", name="Bps")
                        for t in range(DC):
                            nc.tensor.matmul(
                                B[:],
                                ktb[:, t, gi * 128 : (gi + 1) * 128],
                                qt_s[:, t, :],
                                start=(t == 0),
                                stop=(t == DC - 1),
                            )
                        A = psA.tile([128, M], f32, tag="A", name="Aps")
                        for t in range(DC):
                            nc.tensor.matmul(
                                A[:],
                                ktb[:, t, gi * 128 : (gi + 1) * 128],
                                qct_s[:, t, :],
                                start=(t == 0),
                                stop=(t == DC - 1),
                            )
                        am_t = mp.tile([128, M], mybir.dt.uint8, tag="am", name="am_t")
                        lm_t = mp.tile([128, M], mybir.dt.uint8, tag="lm", name="lm_t")
                        nc.sync.dma_start(am_t[:], amh.ap()[g])
                        nc.sync.dma_start(lm_t[:], lmh.ap()[g])
                        t3 = tpool.tile([128, M], f32, tag="t3", name="t3")
                        nc.vector.scalar_tensor_tensor(
                            t3[:], A[:], -bias_val, lm_t[:], ALU.is_gt, ALU.mult
                        )
                        nc.vector.tensor_tensor(t3[:], t3[:], am_t[:], ALU.add)
                        comb = tpool.tile([128, M], f32, tag="comb", name="comb")
                        nc.vector.scalar_tensor_tensor(
                            comb[:], t3[:], MSCALE, B[:], ALU.mult, ALU.add
                        )
                        # -20000 = the (am+st*lm-2) shift, folded into the exp bias
                        nc.scalar.activation(
                            E3[g][:], comb[:], AF.Exp, scale=1.0 / 32.0, bias=shift_s[:, 0:1]
                        )
            q_stack.close()  # qt/qct + ktb SBUF released before O phase

            # ---------------- O phase: E @ v, sums, blend ----------------
            with (
                tc.tile_pool(name="o_out", bufs=4) as opool,
                tc.tile_pool(name="o_xm", bufs=1) as xmp,
                tc.tile_pool(name="o_ps", bufs=1, space="PSUM") as psO,
            ):
                # xm = (1-boundary)*x for all 8 chunks, off the critical path
                xm_t = {}
                for mt in range(MT):
                    for dh in range(2):
                        xm = xmp.tile([128, 512], f32, name=f"xm_{mt}_{dh}")
                        nc.vector.tensor_scalar(
                            xm[:],
                            xn_s[:, mt, dh * 512 : (dh + 1) * 512],
                            omb_s[:, mt : mt + 1],
                            None,
                            ALU.mult,
                        )
                        xm_t[(mt, dh)] = xm
                for dh in range(2):
                    O_ps = [
                        psO.tile([128, 512], f32, tag="O", name=f"O{dh}_{mt}", bufs=4)
                        for mt in range(MT)
                    ]
                    if dh == 0:
                        S_ps = [
                            psO.tile([128, 8], f32, tag="Ssum", name=f"S{mt}", bufs=4)
                            for mt in range(MT)
                        ]
                    for j in range(NCORES):
                        vt = vt_pre.pop((dh, j), None) or load_vt(dh, j)
                        vt_pre.pop((dh, j), None)
                        for gi in range(4):
                            g = j * 4 + gi
                            for mt in range(MT):
                                nc.tensor.matmul(
                                    O_ps[mt][:],
                                    E3[g][:, mt * 128 : (mt + 1) * 128],
                                    vt[:, gi, :],
                                    start=(g == 0),
                                    stop=(g == G - 1),
                                )
                                if dh == 0:
                                    nc.tensor.matmul(
                                        S_ps[mt][:],
                                        E3[g][:, mt * 128 : (mt + 1) * 128],
                                        ones_s[:],
                                        start=(g == 0),
                                        stop=(g == G - 1),
                                    )
                    for mt in range(MT):
                        if dh == 0:
                            nc.vector.reciprocal(
                                recip_s[:, mt : mt + 1], S_ps[mt][:, 0:1]
                            )
                            nc.vector.tensor_tensor(
                                s1_s[:, mt : mt + 1],
                                recip_s[:, mt : mt + 1],
                                bnd_s[:, mt : mt + 1],
                                ALU.mult,
                            )
                        # out = O*(boundary/sumexp) + (1-boundary)*x, one DVE op
                        ot_t = opool.tile([128, 512], f32, tag="ot", name="ot_t")
                        nc.vector.scalar_tensor_tensor(
                            ot_t[:], O_ps[mt][:], s1_s[:, mt : mt + 1],
                            xm_t[(mt, dh)][:], ALU.mult, ALU.add,
                        )
                        nc.sync.dma_start(
                            out.ap()[mt, :, dh * 512 : (dh + 1) * 512], ot_t[:]
                        )
            o_stack.close()

    nc.compile()
    return nc


def make_in_maps(x, attention_mask, learnable_mask, boundary_mask,
                 W_q, b_q, W_k, b_k, W_v, b_v, connection):
    x = np.asarray(x, np.float32)
    amh_full = np.asarray(attention_mask, np.float32).astype(np.uint8)
    lmh_full = np.asarray(learnable_mask, np.float32).astype(np.uint8)
    boundary = np.asarray(boundary_mask, np.float32).reshape(N)
    wqt_h = np.ascontiguousarray(np.asarray(W_q, np.float32).T).reshape(DC, 128, D)
    wkt_h = np.ascontiguousarray(np.asarray(W_k, np.float32).T).reshape(DC, 128, D)
    wvt_h = np.ascontiguousarray(np.asarray(W_v, np.float32).T).reshape(DC, 128, D)
    cn_h = np.ascontiguousarray(np.asarray(connection, np.float32)).reshape(DC, 128, D)
    bq_h = np.ascontiguousarray(np.asarray(b_q, np.float32).reshape(DC, 128).T)
    bk_h = np.ascontiguousarray(np.asarray(b_k, np.float32).reshape(DC, 128).T)
    bv_h = np.ascontiguousarray(np.asarray(b_v, np.float32).reshape(1, D))
    in_maps = []
    for c in range(NCORES):
        rows = slice(c * M, (c + 1) * M)
        in_maps.append(dict(
            xt=np.ascontiguousarray(x[rows].T).reshape(DC, 128, M),
            xn=np.ascontiguousarray(x[rows]).reshape(MT, 128, D),
            wqt=wqt_h, wkt=wkt_h, wvt=wvt_h, cn=cn_h,
            bq=bq_h, bk=bk_h, bv=bv_h,
            bnd=np.ascontiguousarray(boundary[rows]).reshape(MT, 128, 1),
            amh=np.ascontiguousarray(amh_full[rows].T).reshape(G, 128, M),
            lmh=np.ascontiguousarray(lmh_full[rows].T).reshape(G, 128, M),
            ones8=np.ones((128, 8), dtype=ml_dtypes.bfloat16),
            ones1=np.ones((1, 128), dtype=np.float32),
        ))
    return in_maps


_cache = {}


def kernel(x, attention_mask, learnable_mask, boundary_mask,
           W_q, b_q, W_k, b_k, W_v, b_v, connection, bias):
    bias_val = float(np.asarray(bias).reshape(-1)[0])
    if bias_val not in _cache:
        _cache[bias_val] = build(bias_val)
    nc = _cache[bias_val]
    in_maps = make_in_maps(x, attention_mask, learnable_mask, boundary_mask,
                           W_q, b_q, W_k, b_k, W_v, b_v, connection)
    res = bass_utils.run_bass_kernel_spmd(nc, in_maps, core_ids=list(range(NCORES)))
    outs = [res.results[c]["out"].reshape(M, D) for c in range(NCORES)]
    return np.concatenate(outs, axis=0).astype(np.float32)



# revision 10
# speedup vs baseline: 1.3212x; 1.3212x over previous
"""Trainium2 Bass kernel for nn_Attention_75093208203309 (sparse attention).

Contract: kernel(**inputs) takes FULL unsharded inputs (numpy), returns the
FULL [4096, 1024] float32 output. Internally shards query rows across 8
NeuronCores; k/v are computed locally per-core and all-gathered on-device.

Layout strategy (all transposes done host-side in numpy):
  - Per core i (rows = 512*i .. 512*(i+1)):
      qT, qcT   [D, 512]   computed on device from xT shard (f32r matmuls)
      kT_local  [D, 512] bf16 -> 2 chunked AllGathers (D-halves) -> zk0/zk1
      v_local   [512, D] bf16 -> 2 chunked AllGathers (d-halves) -> zv0/zv1
      Chunked gathers issue as soon as each half is computed, so the
        collectives overlap the remaining v/q/qc projections.
      S.T tiles [mk=128, m=512] = k @ qT : lhsT = bf16 kT slice (shared
        stationary operand with conn.T = k @ qcT), rhs = qT / qcT (f32r).
      uint8 masks decoded on DVE; the straight-through hard-sigmoid is a
        single is_gt against -bias on the raw conn logits; exp uses a fixed
        -20000 shift (class-2 mask entries dominate every row, so no row max
        is needed); softmax normalization applied after O = E @ v using
        per-partition reciprocal sums. E and v are bf16.
"""

import contextlib

import numpy as np
import ml_dtypes  # noqa: F401  (np bfloat16 views)

import concourse.bass as bass
import concourse.bacc as bacc
import concourse.mybir as mybir
import concourse.tile as tile
from concourse import bass_utils

f32 = mybir.dt.float32
f32r = mybir.dt.float32r
bf16 = mybir.dt.bfloat16
AF = mybir.ActivationFunctionType
ALU = mybir.AluOpType

NCORES = 8
N, D = 4096, 1024
M = N // NCORES          # 512 rows per core
MT = M // 128            # 4 m-tiles
G = N // 128             # 32 mk-tiles
DC = D // 128            # 8 contraction tiles
HC = DC // 2             # 4 tiles per gather chunk
MSCALE = 320000.0        # 10000 * 32 (folds softmax scale 1/sqrt(D)=1/32)
RG = [list(range(NCORES))]


def build(bias_val: float, timing_mode: bool = False, repeats: int = 1,
          serial: bool = False):
    """timing_mode: single-core variant with zk/zv as ExternalInputs and no
    collectives, for TimelineSim cost-model profiling.
    serial: share the collective DRAM tiles across repeats so reps cannot
    pipeline through the collectives — an R-unroll proxy for the exposed
    single-shot collective latency."""
    nc = bacc.Bacc(None, num_devices=NCORES, debug=False)

    xt = nc.dram_tensor("xt", [DC, 128, M], f32, kind="ExternalInput")
    xn = nc.dram_tensor("xn", [MT, 128, D], f32, kind="ExternalInput")
    wqt = nc.dram_tensor("wqt", [DC, 128, D], f32, kind="ExternalInput")
    wkt = nc.dram_tensor("wkt", [DC, 128, D], f32, kind="ExternalInput")
    wvt = nc.dram_tensor("wvt", [DC, 128, D], f32, kind="ExternalInput")
    cn = nc.dram_tensor("cn", [DC, 128, D], bf16, kind="ExternalInput")
    bq = nc.dram_tensor("bq", [128, DC], f32, kind="ExternalInput")
    bk = nc.dram_tensor("bk", [128, DC], f32, kind="ExternalInput")
    bv = nc.dram_tensor("bv", [1, D], f32, kind="ExternalInput")
    bnd = nc.dram_tensor("bnd", [MT, 128, 1], f32, kind="ExternalInput")
    amh = nc.dram_tensor("amh", [G, 128, M], mybir.dt.uint8, kind="ExternalInput")
    lmh = nc.dram_tensor("lmh", [G, 128, M], mybir.dt.uint8, kind="ExternalInput")
    ones8 = nc.dram_tensor("ones8", [128, 8], mybir.dt.bfloat16, kind="ExternalInput")
    ones1 = nc.dram_tensor("ones1", [1, 128], f32, kind="ExternalInput")
    out = nc.dram_tensor("out", [MT, 128, D], f32, kind="ExternalOutput")

    with tile.TileContext(nc) as tc, contextlib.ExitStack() as ST:
        pp = ST.enter_context(tc.tile_pool(name="persist", bufs=1))
        dp = ST.enter_context(tc.tile_pool(name="dram", bufs=1, space="DRAM"))

        ones_s = pp.tile([128, 8], bf16, name="ones_s")
        onesk1 = pp.tile([1, 128], f32r, name="onesk1")
        bq_s = pp.tile([128, DC], f32, name="bq_s")
        bk_s = pp.tile([128, DC], f32, name="bk_s")
        bv_s = pp.tile([1, D], f32r, name="bv_s")
        bnd_s = pp.tile([128, MT], f32, name="bnd_s")
        recip_s = pp.tile([128, MT], f32, name="recip_s")
        s1_s = pp.tile([128, MT], f32, name="s1_s")
        omb_s = pp.tile([128, MT], f32, name="omb_s")
        shift_s = pp.tile([128, 1], f32, name="shift_s")
        nc.vector.memset(shift_s[:], -20000.0)

        nc.sync.dma_start(ones_s[:], ones8.ap())
        nc.sync.dma_start(onesk1[:], ones1.ap().bitcast(f32r))
        nc.sync.dma_start(bq_s[:], bq.ap())
        nc.sync.dma_start(bk_s[:], bk.ap())
        nc.sync.dma_start(bv_s[:], bv.ap().bitcast(f32r))
        for mt in range(MT):
            nc.sync.dma_start(bnd_s[:, mt : mt + 1], bnd.ap()[mt])
        nc.vector.tensor_scalar(omb_s[:], bnd_s[:], -1.0, 1.0, ALU.mult, ALU.add)

        if timing_mode:
            zk = [
                nc.dram_tensor(f"zk{h}", [NCORES, HC, 128, M], bf16,
                               kind="ExternalInput").ap()
                for h in range(2)
            ]
            zv = [
                nc.dram_tensor(f"zv{h}", [NCORES, MT, 128, 512], bf16,
                               kind="ExternalInput").ap()
                for h in range(2)
            ]
        elif serial:
            # Sharing the collective INPUT buffers across reps serializes the
            # collective chain (WAR: rep r's stores wait on rep r-1's gather
            # reads) — a proxy for exposed single-shot collective latency.
            # Shared outputs must stay per-rep (single-writer constraint).
            kt_loc = [dp.tile([HC, 128, M], bf16, name=f"ktl{h}") for h in range(2)]
            v_loc = [dp.tile([MT, 128, 512], bf16, name=f"vl{h}") for h in range(2)]

        for _rep in range(repeats):
            if not timing_mode:
                zk = [
                    dp.tile([NCORES, HC, 128, M], bf16, name=f"zk{h}_{_rep}",
                            addr_space="Shared")
                    for h in range(2)
                ]
                zv = [
                    dp.tile([NCORES, MT, 128, 512], bf16, name=f"zv{h}_{_rep}",
                            addr_space="Shared")
                    for h in range(2)
                ]
            if not serial:
                kt_loc = [
                    dp.tile([HC, 128, M], bf16, name=f"ktl{h}_{_rep}")
                    for h in range(2)
                ]
                v_loc = [
                    dp.tile([MT, 128, 512], bf16, name=f"vl{h}_{_rep}")
                    for h in range(2)
                ]
            E3 = [
                pp.tile([128, M], bf16, tag="E3", name=f"E3_{g}_{_rep}", bufs=G)
                for g in range(G)
            ]
            # pools whose lifetimes cross phase boundaries, closed manually
            q_stack = contextlib.ExitStack()
            qp = q_stack.enter_context(tc.tile_pool(name="qpool", bufs=1))
            kp = q_stack.enter_context(tc.tile_pool(name="s_kt", bufs=3))
            qt_s = qp.tile([128, DC, M], bf16, name="qt_s")
            qct_s = qp.tile([128, DC, M], bf16, name="qct_s")

            ktb_pre = {}

            def load_ktb(j):
                ktb = kp.tile([128, DC, M], bf16, tag="kt", name="ktb")
                for h in range(2):
                    nc.sync.dma_start(
                        ktb[:, h * HC : (h + 1) * HC, :],
                        zk[h][j].rearrange("t p m -> p t m"),
                    )
                ktb_pre[j] = ktb
                return ktb

            # ---------------- QKV projections (t-outer) ----------------
            with (
                tc.tile_pool(name="qkv_w", bufs=3) as wp,
                tc.tile_pool(name="qkv_x", bufs=1) as xp,
                tc.tile_pool(name="qkv_sb", bufs=3) as sp,
                tc.tile_pool(name="qkv_ps", bufs=8, space="PSUM") as ps1,
            ):
                xt_s = xp.tile([128, DC, M], f32r, name="xt_s")
                for t in range(DC):
                    nc.sync.dma_start(xt_s[:, t, :], xt.ap()[t].bitcast(f32r))

                def load_w_half(wdram, half, name, dt=f32r, tag="w"):
                    w_h = wp.tile([128, DC, 512], dt, tag=tag, name=f"w_{name}{half}")
                    for t in range(DC):
                        src = wdram.ap()[t][:, half * 512 : (half + 1) * 512]
                        if dt == f32r:
                            src = src.bitcast(f32r)
                        nc.sync.dma_start(w_h[:, t, :], src)
                    return w_h

                def mm_half(w_h, rhs_tile, psums):
                    for t in range(DC):
                        for oi in range(4):
                            nc.tensor.matmul(
                                psums[oi][:],
                                w_h[:, t, oi * 128 : (oi + 1) * 128],
                                rhs_tile[:, t, :],
                                start=(t == 0),
                                stop=(t == DC - 1),
                            )

                # kT first: it feeds the first (chunked) all-gather. Each
                # phase's weight DMAs are emitted before the previous phase's
                # store epilogue so store deps never head-of-line-block the
                # weight stream. Each kT half gathers as soon as it's stored.
                wk_h = [load_w_half(wkt, h, "k") for h in range(2)]
                kpss = []
                for half in range(2):
                    kps = [
                        ps1.tile([128, M], f32, tag="ps1", name=f"kps{half}{i}")
                        for i in range(4)
                    ]
                    mm_half(wk_h[half], xt_s, kps)
                    kpss.append(kps)
                wv_h = [load_w_half(wvt, h, "v") for h in range(2)]
                for half in range(2):
                    for oi in range(4):
                        ot = half * 4 + oi
                        kt_sb = sp.tile([128, M], bf16, tag="kvsb", name="kt_sb")
                        nc.scalar.activation(
                            kt_sb[:], kpss[half][oi][:], AF.Identity,
                            bias=bk_s[:, ot : ot + 1],
                        )
                        nc.sync.dma_start(kt_loc[half][oi], kt_sb[:])
                    if not timing_mode:
                        nc.gpsimd.collective_compute(
                            "AllGather", ALU.bypass, replica_groups=RG,
                            ins=[kt_loc[half][:].opt()], outs=[zk[half][:].opt()],
                        )
                load_ktb(0)

                # v: halves are the d-halves directly; gather each as stored
                vpss = []
                for dh in range(2):
                    vps = [
                        ps1.tile([128, 512], f32, tag="ps1", name=f"vps{dh}{mt}")
                        for mt in range(MT)
                    ]
                    for t in range(DC):
                        for mt in range(MT):
                            nc.tensor.matmul(
                                vps[mt][:],
                                xt_s[:, t, mt * 128 : (mt + 1) * 128],
                                wv_h[dh][:, t, :],
                                start=(t == 0),
                                stop=False,
                            )
                    vpss.append(vps)
                wq_h = [load_w_half(wqt, h, "q") for h in range(2)]
                for dh in range(2):
                    for mt in range(MT):
                        nc.tensor.matmul(
                            vpss[dh][mt][:],
                            onesk1[:, :],
                            bv_s[:, dh * 512 : (dh + 1) * 512],
                            start=False,
                            stop=True,
                        )
                        v_sb = sp.tile([128, 512], bf16, tag="kvsb", name="v_sb")
                        nc.scalar.copy(v_sb[:], vpss[dh][mt][:])
                        nc.sync.dma_start(v_loc[dh][mt], v_sb[:])
                    if not timing_mode:
                        nc.gpsimd.collective_compute(
                            "AllGather", ALU.bypass, replica_groups=RG,
                            ins=[v_loc[dh][:].opt()], outs=[zv[dh][:].opt()],
                        )

                qpss = []
                for half in range(2):
                    qps = [
                        ps1.tile([128, M], f32, tag="ps1", name=f"qps{half}{i}")
                        for i in range(4)
                    ]
                    mm_half(wq_h[half], xt_s, qps)
                    qpss.append(qps)
                cn_h = [load_w_half(cn, h, "c", dt=bf16, tag="wc") for h in range(2)]
                for half in range(2):
                    for oi in range(4):
                        ot = half * 4 + oi
                        nc.scalar.activation(
                            qt_s[:, ot, :], qpss[half][oi][:], AF.Identity,
                            bias=bq_s[:, ot : ot + 1],
                        )

                for half in range(2):
                    cps = [
                        ps1.tile([128, M], f32, tag="ps1", name=f"cps{half}{i}")
                        for i in range(4)
                    ]
                    mm_half(cn_h[half], qt_s, cps)
                    for oi in range(4):
                        ot = half * 4 + oi
                        nc.scalar.copy(qct_s[:, ot, :], cps[oi][:])

            # v tiles + xn survive into the O phase
            o_stack = contextlib.ExitStack()
            vpool = o_stack.enter_context(
                tc.tile_pool(name="o_v", bufs=3, side="right")
            )
            xop = o_stack.enter_context(
                tc.tile_pool(name="o_x", bufs=1, side="right")
            )
            xn_s = xop.tile([128, MT, D], f32, name="xn_s")
            nc.sync.dma_start(xn_s[:], xn.ap().rearrange("m p d -> p m d"))
            vt_pre = {}

            def load_vt(dh, j):
                vt = vpool.tile([128, 4, 512], bf16, tag="v", name="vt", bufs=4)
                for vb in range(4):
                    nc.sync.dma_start(vt[:, vb, :], zv[dh][j][vb])
                vt_pre[(dh, j)] = vt
                return vt

            # ---------------- S phase: logits, mask, exp ----------------
            with (
                tc.tile_pool(name="s_m", bufs=12) as mp,
                tc.tile_pool(name="s_t", bufs=4) as tpool,
                tc.tile_pool(name="s_psA", bufs=3, space="PSUM") as psA,
                tc.tile_pool(name="s_psB", bufs=3, space="PSUM") as psB,
            ):
                load_vt(0, 0)
                for j in range(NCORES):
                    ktb = ktb_pre.pop(j, None) or load_ktb(j)
                    ktb_pre.pop(j, None)
                    for gi in range(4):
                        g = j * 4 + gi
                        B = psB.tile([128, M], f32, tag="B", name="Bps")
                        for t in range(DC):
                            nc.tensor.matmul(
                                B[:],
                                ktb[:, t, gi * 128 : (gi + 1) * 128],
                                qt_s[:, t, :],
                                start=(t == 0),
                                stop=(t == DC - 1),
                            )
                        A = psA.tile([128, M], f32, tag="A", name="Aps")
                        for t in range(DC):
                            nc.tensor.matmul(
                                A[:],
                                ktb[:, t, gi * 128 : (gi + 1) * 128],
                                qct_s[:, t, :],
                                start=(t == 0),
                                stop=(t == DC - 1),
                            )
                        am_t = mp.tile([128, M], mybir.dt.uint8, tag="am", name="am_t")
                        lm_t = mp.tile([128, M], mybir.dt.uint8, tag="lm", name="lm_t")
                        nc.sync.dma_start(am_t[:], amh.ap()[g])
                        nc.sync.dma_start(lm_t[:], lmh.ap()[g])
                        t3 = tpool.tile([128, M], f32, tag="t3", name="t3")
                        nc.vector.scalar_tensor_tensor(
                            t3[:], A[:], -bias_val, lm_t[:], ALU.is_gt, ALU.mult
                        )
                        nc.vector.tensor_tensor(t3[:], t3[:], am_t[:], ALU.add)
                        comb = tpool.tile([128, M], f32, tag="comb", name="comb")
                        nc.vector.scalar_tensor_tensor(
                            comb[:], t3[:], MSCALE, B[:], ALU.mult, ALU.add
                        )
                        # -20000 = the (am+st*lm-2) shift, folded into the exp bias
                        nc.scalar.activation(
                            E3[g][:], comb[:], AF.Exp, scale=1.0 / 32.0, bias=shift_s[:, 0:1]
                        )
            q_stack.close()  # qt/qct + ktb SBUF released before O phase

            # ---------------- O phase: E @ v, sums, blend ----------------
            with (
                tc.tile_pool(name="o_out", bufs=4) as opool,
                tc.tile_pool(name="o_xm", bufs=1) as xmp,
                tc.tile_pool(name="o_ps", bufs=1, space="PSUM") as psO,
            ):
                # xm = (1-boundary)*x for all 8 chunks, off the critical path
                xm_t = {}
                for mt in range(MT):
                    for dh in range(2):
                        xm = xmp.tile([128, 512], f32, name=f"xm_{mt}_{dh}")
                        nc.vector.tensor_scalar(
                            xm[:],
                            xn_s[:, mt, dh * 512 : (dh + 1) * 512],
                            omb_s[:, mt : mt + 1],
                            None,
                            ALU.mult,
                        )
                        xm_t[(mt, dh)] = xm
                for dh in range(2):
                    O_ps = [
                        psO.tile([128, 512], f32, tag="O", name=f"O{dh}_{mt}", bufs=4)
                        for mt in range(MT)
                    ]
                    if dh == 0:
                        S_ps = [
                            psO.tile([128, 8], f32, tag="Ssum", name=f"S{mt}", bufs=4)
                            for mt in range(MT)
                        ]
                    for j in range(NCORES):
                        vt = vt_pre.pop((dh, j), None) or load_vt(dh, j)
                        vt_pre.pop((dh, j), None)
                        for gi in range(4):
                            g = j * 4 + gi
                            for mt in range(MT):
                                nc.tensor.matmul(
                                    O_ps[mt][:],
                                    E3[g][:, mt * 128 : (mt + 1) * 128],
                                    vt[:, gi, :],
                                    start=(g == 0),
                                    stop=(g == G - 1),
                                )
                                if dh == 0:
                                    nc.tensor.matmul(
                                        S_ps[mt][:],
                                        E3[g][:, mt * 128 : (mt + 1) * 128],
                                        ones_s[:],
                                        start=(g == 0),
                                        stop=(g == G - 1),
                                    )
                    for mt in range(MT):
                        if dh == 0:
                            nc.vector.reciprocal(
                                recip_s[:, mt : mt + 1], S_ps[mt][:, 0:1]
                            )
                            nc.vector.tensor_tensor(
                                s1_s[:, mt : mt + 1],
                                recip_s[:, mt : mt + 1],
                                bnd_s[:, mt : mt + 1],
                                ALU.mult,
                            )
                        # out = O*(boundary/sumexp) + (1-boundary)*x, one DVE op
                        ot_t = opool.tile([128, 512], f32, tag="ot", name="ot_t")
                        nc.vector.scalar_tensor_tensor(
                            ot_t[:], O_ps[mt][:], s1_s[:, mt : mt + 1],
                            xm_t[(mt, dh)][:], ALU.mult, ALU.add,
                        )
                        nc.sync.dma_start(
                            out.ap()[mt, :, dh * 512 : (dh + 1) * 512], ot_t[:]
                        )
            o_stack.close()

    nc.compile()
    return nc


def make_in_maps(x, attention_mask, learnable_mask, boundary_mask,
                 W_q, b_q, W_k, b_k, W_v, b_v, connection):
    x = np.asarray(x, np.float32)
    amh_full = np.asarray(attention_mask, np.float32).astype(np.uint8)
    lmh_full = np.asarray(learnable_mask, np.float32).astype(np.uint8)
    boundary = np.asarray(boundary_mask, np.float32).reshape(N)
    wqt_h = np.ascontiguousarray(np.asarray(W_q, np.float32).T).reshape(DC, 128, D)
    wkt_h = np.ascontiguousarray(np.asarray(W_k, np.float32).T).reshape(DC, 128, D)
    wvt_h = np.ascontiguousarray(np.asarray(W_v, np.float32).T).reshape(DC, 128, D)
    cn_h = np.ascontiguousarray(
        np.asarray(connection, np.float32).astype(ml_dtypes.bfloat16)
    ).reshape(DC, 128, D)
    bq_h = np.ascontiguousarray(np.asarray(b_q, np.float32).reshape(DC, 128).T)
    bk_h = np.ascontiguousarray(np.asarray(b_k, np.float32).reshape(DC, 128).T)
    bv_h = np.ascontiguousarray(np.asarray(b_v, np.float32).reshape(1, D))
    in_maps = []
    for c in range(NCORES):
        rows = slice(c * M, (c + 1) * M)
        in_maps.append(dict(
            xt=np.ascontiguousarray(x[rows].T).reshape(DC, 128, M),
            xn=np.ascontiguousarray(x[rows]).reshape(MT, 128, D),
            wqt=wqt_h, wkt=wkt_h, wvt=wvt_h, cn=cn_h,
            bq=bq_h, bk=bk_h, bv=bv_h,
            bnd=np.ascontiguousarray(boundary[rows]).reshape(MT, 128, 1),
            amh=np.ascontiguousarray(amh_full[rows].T).reshape(G, 128, M),
            lmh=np.ascontiguousarray(lmh_full[rows].T).reshape(G, 128, M),
            ones8=np.ones((128, 8), dtype=ml_dtypes.bfloat16),
            ones1=np.ones((1, 128), dtype=np.float32),
        ))
    return in_maps


_cache = {}


def kernel(x, attention_mask, learnable_mask, boundary_mask,
           W_q, b_q, W_k, b_k, W_v, b_v, connection, bias):
    bias_val = float(np.asarray(bias).reshape(-1)[0])
    if bias_val not in _cache:
        _cache[bias_val] = build(bias_val)
    nc = _cache[bias_val]
    in_maps = make_in_maps(x, attention_mask, learnable_mask, boundary_mask,
                           W_q, b_q, W_k, b_k, W_v, b_v, connection)
    res = bass_utils.run_bass_kernel_spmd(nc, in_maps, core_ids=list(range(NCORES)))
    outs = [res.results[c]["out"].reshape(M, D) for c in range(NCORES)]
    return np.concatenate(outs, axis=0).astype(np.float32)
